# revision 1
# baseline (speedup 1.0000x reference)
"""Trainium2 Bass kernel for causal self-attention (B=2, S=2048, D=1024, H=16).

Sharding: 8 cores = 2 batch groups x 4 head-groups (tensor parallel).
Core c handles batch b = c // 4 and heads [4*(c%4), 4*(c%4)+4).
Each core computes a partial out-projection [S, D]; the host sums the 4
partials of each batch group (row-parallel TP unshard) and adds bout.

Per-core pipeline (all layouts chosen so no on-device transposes of
activations are needed except small V blocks):
  1. qkvT[col, s] = Wqkv_local.T @ x.T   (x passed pre-transposed, a host
     layout choice; weights are naturally [D, cols] = lhsT layout)
  2. scoresT[k, q] = K_h^T.T @ Q_h per 128-wide k-chunk, causal blocks only.
     Key-padding mask + 1/sqrt(64) scale fold into the ACT exp (per-partition
     bias = per-k bias in this transposed layout).  P = exp(scores') in bf16.
  3. attT[65, q] = V_ext^T @ P  where V_ext = [V_h | ones]: row 64 is the
     softmax denominator.  No separate reduction needed.
  4. normalize per-q: recip = 1/(den + eps) replicated via a K=1 matmul;
     att_n = attT * recip; query-padding mask applied as one big multiply.
  5. out_partial[s, :] = att_n.T @ Wout_local  (att_n is already the lhsT
     layout needed), DMA PSUM -> DRAM directly.
"""

import os
import sys

import numpy as np

for _p in ("/opt/trn_rl_repo",):
    if _p not in sys.path and os.path.isdir(_p):
        sys.path.insert(0, _p)

import concourse.bass as bass
import concourse.mybir as mybir
from concourse import tile
from concourse.bass_utils import run_bass_kernel_spmd

B, S, D, H = 2, 2048, 1024, 16
HD = D // H  # 64
HEADS_PER_CORE = 4
CORES = 8
LOCAL_COLS = 3 * HEADS_PER_CORE * HD  # 768 (q|k|v for 4 heads)
NEG = -1.0e30
EPS = 1.0e-9  # within ACT-reciprocal valid range +-[2^-42, 2^42]

F32 = mybir.dt.float32
F32R = mybir.dt.float32r
BF16 = mybir.dt.bfloat16

AF = mybir.ActivationFunctionType

N_STILE = 4  # 512-wide s tiles
N_KCH = S // 128  # 16 k-chunks
VEXT_W = HEADS_PER_CORE * (HD + 1)  # 260


def round_f32r(a):
    """Round fp32 array to fp32r (11-bit mantissa, round-to-nearest-even)."""
    u = np.ascontiguousarray(a, np.float32).view(np.uint32)
    low = u & np.uint32(0x00000FFF)
    base = u & np.uint32(0xFFFFF000)
    lsb = (u >> np.uint32(12)) & np.uint32(1)
    up = (low > 0x800) | ((low == 0x800) & (lsb == 1))
    return (base + (up.astype(np.uint32) << np.uint32(12))).view(np.float32)



def _split_waits(nc, cap=1):
    """Walrus in this container allows few sync-waits per instruction.
    Hoist excess waits onto preceding same-engine NoOps (same sequencer,
    program order => semantics preserved).  fp32-path Matmult lowers to
    LDW+MM whose LW struct takes no waits at all -> cap 0."""
    uid = [0]
    for fn in nc.m.functions:
        for bb in fn.blocks:
            insts = bb.instructions
            out = []
            for ins in insts:
                icap = 0 if isinstance(ins, mybir.InstMatmult) else cap
                si = ins.sync_info
                waits = list(si.on_wait) if (si and si.on_wait) else []
                if len(waits) > icap:
                    extra = waits[:-icap] if icap else waits
                    keep = waits[-icap:] if icap else []
                    gcap = max(cap, 1)
                    for i in range(0, len(extra), gcap):
                        grp = extra[i : i + gcap]
                        nop = mybir.InstNoOp(
                            name=f"wsplit-{uid[0]}", ins=[], outs=[]
                        )
                        uid[0] += 1
                        nop.engine = ins.engine
                        nop.sync_info = mybir.SyncInfo(on_wait=grp, on_update=[])
                        out.append(nop)
                    si.on_wait = keep
                out.append(ins)
            if len(out) != len(insts):
                insts[:] = out
    return nc


# score-chunk table: per tj (= j//4), list of (start_col, width) chunks of
# the valid q-range [512*tj, 2048), each <= 1024 wide, 512-aligned pieces
CHUNKS = {
    0: [(0, 1024), (1024, 1024)],
    1: [(512, 512), (1024, 1024)],
    2: [(1024, 1024)],
    3: [(1536, 512)],
}


def _chunk_for(tj, col):
    for cs, cw in CHUNKS[tj]:
        if cs <= col < cs + cw:
            return cs, cw
    raise ValueError((tj, col))


def _act_recip(nc, out_ap, in_ap):
    """ACT-engine reciprocal (bass blocks ActivationFunctionType.Reciprocal
    behind an accuracy warning; ~1e-5 rel err is fine for this kernel and it
    replaces a 53us DVE InstReciprocal with one ~2us ACTIVATE)."""
    eng = nc.scalar
    inputs = [eng.lower_ap(in_ap)]
    for v in (0.0, 1.0, 0.0):  # bias, scale, alpha
        inputs.append(mybir.ImmediateValue(dtype=mybir.dt.float32, value=v))
    return eng.add_instruction(
        mybir.InstActivation(
            name=eng.bass.get_next_instruction_name(),
            func=mybir.ActivationFunctionType.Reciprocal,
            ins=inputs,
            outs=[eng.lower_ap(out_ap)],
        )
    )


def build_nc(mm_dt="f32r", p_dt="bf16", split_waits=True):
    """Build the SPMD single-core program (same program on all 8 cores)."""
    nc = bass.Bass()
    mdt = F32R if mm_dt == "f32r" else F32
    pdt = BF16 if p_dt == "bf16" else F32
    scale = float(HD) ** -0.5

    xT = nc.dram_tensor("xT", [D, S], mdt, kind="ExternalInput")
    wqkv = nc.dram_tensor("wqkv", [D, LOCAL_COLS], mdt, kind="ExternalInput")
    bqkv_pc = nc.dram_tensor("bqkv_pc", [128, 6], F32, kind="ExternalInput")
    wout = nc.dram_tensor("wout", [256, D], mdt, kind="ExternalInput")
    kbias = nc.dram_tensor("kbias", [128, N_KCH], F32, kind="ExternalInput")
    qmask_rep = nc.dram_tensor("qmask_rep", [128, S], F32, kind="ExternalInput")
    tri = nc.dram_tensor("tri", [128, 128], F32, kind="ExternalInput")
    ident = nc.dram_tensor("ident", [128, 128], pdt, kind="ExternalInput")
    out = nc.dram_tensor("out", [S, D], F32, kind="ExternalOutput")

    with tile.TileContext(nc) as tc:
        with (
            tc.tile_pool(name="consts", bufs=1) as consts,
            tc.tile_pool(name="persist", bufs=1) as persist,
        ):
            # ---- constants / persistent SBUF ----
            wout_sb = consts.tile([128, 2 * D], mdt)
            for ch in range(2):
                nc.sync.dma_start(
                    wout_sb[:, ch * D : (ch + 1) * D],
                    wout[ch * 128 : (ch + 1) * 128, :],
                )
            kbias_sb = consts.tile([128, N_KCH], F32)
            nc.sync.dma_start(kbias_sb[:], kbias[:])
            qmask_sb = consts.tile([128, S], F32)
            nc.sync.dma_start(qmask_sb[:], qmask_rep[:])
            tri_sb = consts.tile([128, 128], F32)
            nc.sync.dma_start(tri_sb[:], tri[:])

            # qkvT: 6 col-chunks x [128, S] in bf16; 0,1 = q, 2,3 = k, 4,5 = v
            qkvT = persist.tile([128, 6 * S], pdt)
            # V_ext: per k-chunk [128, 260]: 4 heads x (64 V cols + ones col)
            v_ext = persist.tile([128, N_KCH * VEXT_W], pdt)
            # att_u: attended (transposed), unnormalized then normalized in place
            att_u = persist.tile([128, 2 * S], mdt)
            # denominators: one row per head at partition h*32 (engine start-
            # partition constraint: must be 0/32/64/96)
            den4 = persist.tile([128, S], F32)
            recip4 = persist.tile([128, S], F32)

            # ==================== Phase A: QKV ====================
            with (
                tc.tile_pool(name="aconsts", bufs=1) as aconsts,
                tc.tile_pool(name="xs", bufs=3) as xs,
                tc.tile_pool(name="qkv_ps", bufs=6, space="PSUM") as qkv_ps,
                tc.tile_pool(name="tr_ps", bufs=2, space="PSUM") as tr_ps,
            ):
                wqkv_sb = aconsts.tile([128, 8 * LOCAL_COLS], mdt)
                for d in range(8):
                    nc.sync.dma_start(
                        wqkv_sb[:, d * LOCAL_COLS : (d + 1) * LOCAL_COLS],
                        wqkv[d * 128 : (d + 1) * 128, :],
                    )
                bqkv_sb = aconsts.tile([128, 6], F32)
                nc.sync.dma_start(bqkv_sb[:], bqkv_pc[:])
                ident_sb = aconsts.tile([128, 128], pdt)
                nc.sync.dma_start(ident_sb[:], ident[:])
                for t in range(N_STILE):
                    ps = [qkv_ps.tile([128, 512], F32, tag="qkvps", name=f"qkvps_{t}_{i}") for i in range(6)]
                    for d in range(8):
                        xt = xs.tile([128, 512], mdt, tag="xs", name=f"xs_{t}_{d}")
                        nc.gpsimd.dma_start(
                            xt[:], xT[d * 128 : (d + 1) * 128, t * 512 : (t + 1) * 512]
                        )
                        for cc in range(6):
                            nc.tensor.matmul(
                                ps[cc][:],
                                wqkv_sb[:, d * LOCAL_COLS + cc * 128 : d * LOCAL_COLS + (cc + 1) * 128],
                                xt[:],
                                start=(d == 0),
                                stop=(d == 7),
                            )
                    for cc in range(6):
                        nc.vector.tensor_scalar_add(
                            qkvT[:, cc * S + t * 512 : cc * S + (t + 1) * 512],
                            ps[cc][:],
                            bqkv_sb[:, cc : cc + 1],
                        )

                # V transposes: vT chunks 4,5 -> V_ext natural layout (+ones)
                for sc in range(N_KCH):
                    base = sc * VEXT_W
                    nc.any.memset(
                        v_ext[:, base : base + VEXT_W].rearrange(
                            "p (h c) -> p h c", h=HEADS_PER_CORE
                        )[:, :, HD : HD + 1],
                        1.0,
                    )
                    for hp in range(2):  # head pairs
                        tp = tr_ps.tile([128, 128], pdt, tag="trps", name=f"trps_{sc}_{hp}")
                        nc.tensor.transpose(
                            tp[:],
                            qkvT[:, (4 + hp) * S + sc * 128 : (4 + hp) * S + (sc + 1) * 128],
                            ident_sb[:],
                        )
                        nc.vector.tensor_copy(
                            v_ext[:, base + hp * 2 * (HD + 1) : base + (hp * 2 + 2) * (HD + 1)]
                            .rearrange("p (h c) -> p h c", h=2)[:, :, 0:HD],
                            tp[:].rearrange("p (h c) -> p h c", h=2),
                        )

            # ==================== Phase B: attention ====================
            with (
                tc.tile_pool(name="sc_ps", bufs=3, space="PSUM") as sc_ps,
                tc.tile_pool(name="av_ps", bufs=2, space="PSUM") as av_ps,
                tc.tile_pool(name="pt", bufs=4) as ptp,
                tc.tile_pool(name="rr", bufs=3) as rrp,
                tc.tile_pool(name="outsb", bufs=2) as outsb,
                tc.tile_pool(name="dram", bufs=1, space="DRAM") as dramp,
            ):
                recip4_dram = dramp.tile([4, S], F32, name="recip4_dram")
                def emit_scores_pair(p, j):
                    qch = p
                    kch = 2 + p
                    tj = j // 4
                    for ci, (cs, cw) in enumerate(CHUNKS[tj]):
                        tiles = []
                        for hh in range(2):
                            h = 2 * p + hh
                            qrow = hh * 64
                            sps = sc_ps.tile(
                                [128, 1024], F32, tag="scps", name=f"scps_{h}_{j}_{ci}"
                            )
                            tiles.append(sps)
                        # alternate heads per 512-slice: adjacent matmuls use
                        # disjoint row groups -> concurrent execution
                        for o in range(0, cw, 512):
                            t = (cs + o) // 512
                            for hh in range(2):
                                qrow = hh * 64
                                nc.tensor.matmul(
                                    tiles[hh][:, o : o + 512],
                                    qkvT[qrow : qrow + 64, kch * S + j * 128 : kch * S + (j + 1) * 128],
                                    qkvT[qrow : qrow + 64, qch * S + t * 512 : qch * S + (t + 1) * 512],
                                    start=True,
                                    stop=True,
                                )
                        for hh in range(2):
                            h = 2 * p + hh
                            sps = tiles[hh]
                            pt = ptp.tile(
                                [128, cw], pdt, tag=f"pt{cw}",
                                bufs=(32 if cw == 1024 else 16),
                                name=f"pt_{h}_{j}_{ci}",
                            )
                            if ci == 0:
                                db = j * 128 - cs  # diag block offset in chunk
                                nc.vector.tensor_add(
                                    sps[:, db : db + 128], sps[:, db : db + 128], tri_sb[:]
                                )
                                if db > 0:
                                    nc.any.memset(pt[:, 0:db], 0.0)
                                nc.scalar.activation(
                                    pt[:, db:cw], sps[:, db:cw], AF.Exp,
                                    bias=kbias_sb[:, j : j + 1], scale=scale,
                                )
                            else:
                                nc.scalar.activation(
                                    pt[:, 0:cw], sps[:, 0:cw], AF.Exp,
                                    bias=kbias_sb[:, j : j + 1], scale=scale,
                                )
                            pts[(h, j, cs)] = pt

                def emit_av_pair(p, t):
                    qch = p
                    jmax = 4 * t + 3
                    for hh in range(2):
                        h = 2 * p + hh
                        qrow = hh * 64
                        aps = av_ps.tile(
                            [65, 512], F32, tag="avps", padded_shape=[128, 512],
                            name=f"avps_{h}_{t}",
                        )
                        for j in range(jmax + 1):
                            tj = j // 4
                            cs, cw = _chunk_for(tj, t * 512)
                            off = t * 512 - cs
                            nc.tensor.matmul(
                                aps[:],
                                v_ext[:, j * VEXT_W + h * (HD + 1) : j * VEXT_W + (h + 1) * (HD + 1)],
                                pts[(h, j, cs)][:, off : off + 512],
                                start=(j == 0),
                                stop=(j == jmax),
                            )
                        nc.vector.tensor_scalar_add(
                            den4[h * 32 : h * 32 + 1, t * 512 : (t + 1) * 512],
                            aps[64:65, :],
                            EPS,
                        )
                        nc.scalar.activation(
                            att_u[qrow : qrow + 64, qch * S + t * 512 : qch * S + (t + 1) * 512],
                            aps[0:64, :],
                            AF.Identity,
                        )

                pts = {}

                def emit_norm_outproj(t):
                    """All 4 heads' denominators for q-tile t are ready:
                    reciprocal + qmask fold + broadcast + normalize + project."""
                    for h in range(HEADS_PER_CORE):
                        _act_recip(
                            nc,
                            recip4[h * 32 : h * 32 + 1, t * 512 : (t + 1) * 512],
                            den4[h * 32 : h * 32 + 1, t * 512 : (t + 1) * 512],
                        )
                        nc.vector.tensor_mul(
                            recip4[h * 32 : h * 32 + 1, t * 512 : (t + 1) * 512],
                            recip4[h * 32 : h * 32 + 1, t * 512 : (t + 1) * 512],
                            qmask_sb[h * 32 : h * 32 + 1, t * 512 : (t + 1) * 512],
                        )
                    nc.sync.dma_start(
                        recip4_dram[:, t * 512 : (t + 1) * 512],
                        recip4[:, t * 512 : (t + 1) * 512]
                        .rearrange("(a b) c -> a b c", b=32)[:, 0:1, :]
                        .rearrange("a b c -> (a b) c"),
                    )
                    for qch in range(2):
                        rr = rrp.tile([128, 512], F32, tag="rr", name=f"rr_{qch}_{t}")
                        for hh in range(2):
                            h = qch * 2 + hh
                            nc.sync.dma_start(
                                rr[hh * 64 : (hh + 1) * 64, :],
                                recip4_dram[h : h + 1, t * 512 : (t + 1) * 512].to_broadcast((64, 512)),
                            )
                        sl = att_u[:, qch * S + t * 512 : qch * S + (t + 1) * 512]
                        nc.vector.tensor_mul(sl, sl, rr[:])
                    for st in range(4 * t, 4 * t + 4):
                        for n in range(2):
                            ops = av_ps.tile([128, 512], F32, tag="avps", name=f"outps_{st}_{n}")
                            for ch in range(2):
                                nc.tensor.matmul(
                                    ops[:],
                                    att_u[:, ch * S + st * 128 : ch * S + (st + 1) * 128],
                                    wout_sb[:, ch * D + n * 512 : ch * D + (n + 1) * 512],
                                    start=(ch == 0),
                                    stop=(ch == 1),
                                )
                            osb = outsb.tile([128, 512], F32, tag="outsb", name=f"outsb_{st}_{n}")
                            nc.vector.tensor_copy(osb[:], ops[:])
                            nc.sync.dma_start(
                                out[st * 128 : (st + 1) * 128, n * 512 : (n + 1) * 512],
                                osb[:],
                            )

                # head-PAIR emission with per-tile AV drains; once pair 1's
                # AV(t) lands, all four heads of q-tile t are complete ->
                # normalize + out-project t while pair 1 continues scoring.
                for p in range(2):
                    for j in range(N_KCH):
                        emit_scores_pair(p, j)
                        if j % 4 == 3:
                            t = j // 4
                            emit_av_pair(p, t)
                            if p == 1:
                                emit_norm_outproj(t)

    return _split_waits(nc) if split_waits else nc


def make_in_maps(x, attention_mask, Wqkv, bqkv, Wout, mm_dt="f32r"):
    """Shard full inputs into the 8 per-core input dicts."""
    rnd = round_f32r if mm_dt == "f32r" else (lambda a: np.ascontiguousarray(a, np.float32))
    x = np.asarray(x, np.float32)
    attention_mask = np.asarray(attention_mask)
    Wqkv = np.asarray(Wqkv, np.float32)
    bqkv = np.asarray(bqkv, np.float32)
    Wout = np.asarray(Wout, np.float32)

    import ml_dtypes

    tri = np.where(
        np.arange(128)[:, None] <= np.arange(128)[None, :], 0.0, NEG
    ).astype(np.float32)
    ident = np.eye(128, dtype=ml_dtypes.bfloat16)

    in_maps = []
    for c in range(CORES):
        b, g = divmod(c, 4)
        cs = 256 * g  # local col start within each of q/k/v blocks
        wq = Wqkv[:, cs : cs + 256]
        wk = Wqkv[:, D + cs : D + cs + 256]
        wv = Wqkv[:, 2 * D + cs : 2 * D + cs + 256]
        w_local = np.ascontiguousarray(np.concatenate([wq, wk, wv], axis=1))
        b_local = np.concatenate(
            [bqkv[cs : cs + 256], bqkv[D + cs : D + cs + 256], bqkv[2 * D + cs : 2 * D + cs + 256]]
        )
        bqkv_pc = np.ascontiguousarray(b_local.reshape(6, 128).T)
        wout_l = np.ascontiguousarray(Wout[cs : cs + 256, :])
        m = attention_mask[b].astype(np.float32)
        kb = np.where(m > 0, 0.0, NEG).astype(np.float32)
        kbias_pc = np.ascontiguousarray(kb.reshape(N_KCH, 128).T)
        qmask_rep = np.ascontiguousarray(np.broadcast_to(m[None, :], (128, S)))
        in_maps.append(
            {
                "xT": rnd(x[b].T),
                "wqkv": rnd(w_local),
                "bqkv_pc": bqkv_pc,
                "wout": rnd(wout_l),
                "kbias": kbias_pc,
                "qmask_rep": qmask_rep,
                "tri": tri,
                "ident": ident,
            }
        )
    return in_maps


_NC_CACHE = {}


def _get_nc(mm_dt="f32r", p_dt="bf16"):
    key = (mm_dt, p_dt)
    if key not in _NC_CACHE:
        _NC_CACHE[key] = build_nc(*key)
    return _NC_CACHE[key]


def kernel(x, attention_mask, Wqkv, bqkv, Wout, bout, _trace=False, _trace_kwargs=None):
    bout = np.asarray(bout, np.float32)
    mm_dt = os.environ.get("ATTN_MM_DT", "f32r")
    p_dt = os.environ.get("ATTN_P_DT", "bf16")
    in_maps = make_in_maps(x, attention_mask, Wqkv, bqkv, Wout, mm_dt=mm_dt)
    nc = _get_nc(mm_dt, p_dt)
    res = run_bass_kernel_spmd(
        nc,
        in_maps,
        list(range(CORES)),
        trace=_trace,
        **(_trace_kwargs or {}),
    )
    outs = [res.results[c]["out"] for c in range(CORES)]
    full = np.empty((B, S, D), np.float32)
    for b in range(B):
        full[b] = outs[4 * b] + outs[4 * b + 1] + outs[4 * b + 2] + outs[4 * b + 3] + bout
    if _trace:
        return full, res
    return full



# revision 10
# speedup vs baseline: 1.3997x; 1.3997x over previous
"""Trainium2 Bass kernel for causal self-attention (B=2, S=2048, D=1024, H=16).

Sharding: 8 cores = 2 batch groups x 4 head-groups (tensor parallel).
Core c handles batch b = c // 4 and heads [4*(c%4), 4*(c%4)+4).
Each core computes a partial out-projection [S, D]; the host sums the 4
partials of each batch group (row-parallel TP unshard) and adds bout.

v2: fully bf16 matmul path, q-tile-major software pipeline (QKV for tile
t+1 overlaps attention for tile t), key-padding mask folded into V_ext
rows + the ones-column (so exp needs no per-chunk bias -> no ACT table
swaps), reciprocal on DVE via reciprocal_approx_fast.

Per-core pipeline per 512-wide q-tile t:
  1. qkvT[col, s] = Wqkv_local.T @ x.T (bf16, cc-major so psum rotates)
  2. V transposes for the 4 new k-chunks; key mask folded in, ones-col
     gets the 0/1 key mask (so the denominator row counts valid keys).
  3. scoresT[k, q] = K_h^T.T @ Q_h per 128-wide k-chunk, pairs of heads
     run concurrently on disjoint PE row groups; tri mask on diagonal
     blocks via DVE; P = exp(scale * scores) in bf16 (no bias).
  4. attT[65, q] = V_ext^T @ P; row 64 is the softmax denominator.
  5. normalize: recip on DVE, qmask fold, broadcast via DRAM round trip,
     one big multiply; out_partial = att_n.T @ Wout_local.
"""

import os
import sys

import numpy as np

for _p in ("/opt/trn_rl_repo",):
    if _p not in sys.path and os.path.isdir(_p):
        sys.path.insert(0, _p)

import concourse.bass as bass
import concourse.mybir as mybir
from concourse import tile
from concourse.bass_utils import run_bass_kernel_spmd

B, S, D, H = 2, 2048, 1024, 16
HD = D // H  # 64
HEADS_PER_CORE = 4
CORES = 8
LOCAL_COLS = 3 * HEADS_PER_CORE * HD  # 768 (q|k|v for 4 heads)
NEG = -1.0e30
EPS = 1.0e-9

F32 = mybir.dt.float32
BF16 = mybir.dt.bfloat16

AF = mybir.ActivationFunctionType

N_TILE = 4  # 512-wide q/s tiles
N_KCH = S // 128  # 16 k-chunks
VEXT_W = HEADS_PER_CORE * (HD + 1)  # 260


def _split_waits(nc, cap=1):
    """Walrus in this container allows few sync-waits per instruction.
    Hoist excess waits onto preceding same-engine NoOps (same sequencer,
    program order => semantics preserved).  fp32-path Matmult lowers to
    LDW+MM whose LW struct takes no waits at all -> cap 0."""
    uid = [0]
    for fn in nc.m.functions:
        for bb in fn.blocks:
            insts = bb.instructions
            out = []
            for ins in insts:
                icap = 0 if isinstance(ins, mybir.InstMatmult) else cap
                si = ins.sync_info
                waits = list(si.on_wait) if (si and si.on_wait) else []
                if len(waits) > icap:
                    extra = waits[:-icap] if icap else waits
                    keep = waits[-icap:] if icap else []
                    gcap = max(cap, 1)
                    for i in range(0, len(extra), gcap):
                        grp = extra[i : i + gcap]
                        nop = mybir.InstNoOp(
                            name=f"wsplit-{uid[0]}", ins=[], outs=[]
                        )
                        uid[0] += 1
                        nop.engine = ins.engine
                        nop.sync_info = mybir.SyncInfo(on_wait=grp, on_update=[])
                        out.append(nop)
                    si.on_wait = keep
                out.append(ins)
            if len(out) != len(insts):
                insts[:] = out
    return nc


def build_nc(split_waits=True):
    """Build the SPMD single-core program (same program on all 8 cores)."""
    nc = bass.Bass()
    scale = float(HD) ** -0.5

    xT = nc.dram_tensor("xT", [D, S], BF16, kind="ExternalInput")
    wqkv = nc.dram_tensor("wqkv", [D, LOCAL_COLS], BF16, kind="ExternalInput")
    bqkv_pc = nc.dram_tensor("bqkv_pc", [128, 6], F32, kind="ExternalInput")
    wout = nc.dram_tensor("wout", [256, D], BF16, kind="ExternalInput")
    kmask = nc.dram_tensor("kmask", [128, N_KCH], F32, kind="ExternalInput")
    qmask_rep = nc.dram_tensor("qmask_rep", [128, S], F32, kind="ExternalInput")
    tri = nc.dram_tensor("tri", [128, 128], F32, kind="ExternalInput")
    ident = nc.dram_tensor("ident", [128, 128], BF16, kind="ExternalInput")
    out = nc.dram_tensor("out", [S, D], F32, kind="ExternalOutput")

    with tile.TileContext(nc) as tc:
        with (
            tc.tile_pool(name="consts", bufs=1) as consts,
            tc.tile_pool(name="persist", bufs=1) as persist,
            tc.tile_pool(name="xs", bufs=2) as xs,
            tc.tile_pool(name="pt", bufs=4) as ptp,
            tc.tile_pool(name="rr", bufs=4) as rrp,
            tc.tile_pool(name="outsb", bufs=3) as outsb,
            tc.tile_pool(name="dram", bufs=1, space="DRAM") as dramp,
            tc.tile_pool(name="gp_ps", bufs=2, space="PSUM") as gp_ps,
            tc.tile_pool(name="sc_ps", bufs=3, space="PSUM") as sc_ps,
            tc.tile_pool(name="av_ps", bufs=2, space="PSUM") as av_ps,
            tc.tile_pool(name="tr_ps", bufs=1, space="PSUM") as tr_ps,
        ):
            # ---- constants / persistent SBUF ----
            wqkv_sb = consts.tile([128, 8 * LOCAL_COLS], BF16)
            for d in range(8):
                nc.sync.dma_start(
                    wqkv_sb[:, d * LOCAL_COLS : (d + 1) * LOCAL_COLS],
                    wqkv[d * 128 : (d + 1) * 128, :],
                )
            wout_sb = consts.tile([128, 2 * D], BF16)
            for ch in range(2):
                nc.sync.dma_start(
                    wout_sb[:, ch * D : (ch + 1) * D],
                    wout[ch * 128 : (ch + 1) * 128, :],
                )
            bqkv_sb = consts.tile([128, 6], F32)
            nc.sync.dma_start(bqkv_sb[:], bqkv_pc[:])
            kmask_sb = consts.tile([128, N_KCH], F32)
            nc.sync.dma_start(kmask_sb[:], kmask[:])
            qmask_sb = consts.tile([128, S], F32)
            nc.sync.dma_start(qmask_sb[:], qmask_rep[:])
            tri_sb = consts.tile([128, 128], F32)
            nc.sync.dma_start(tri_sb[:], tri[:])
            ident_sb = consts.tile([128, 128], BF16)
            nc.sync.dma_start(ident_sb[:], ident[:])

            # qkvT: 6 col-chunks x [128, S]; 0,1 = q, 2,3 = k, 4,5 = v
            qkvT = persist.tile([128, 6 * S], BF16)
            # V_ext per k-chunk [128, 260]: 4 heads x (64 V cols + mask col)
            v_ext = persist.tile([128, N_KCH * VEXT_W], BF16)
            att_u = persist.tile([128, 2 * S], BF16)
            # denominators: one row per head at partition h*32 (engine
            # start-partition constraint); garbage rows preset to 1.0 so the
            # full-width reciprocal stays in range
            den4 = persist.tile([128, S], F32)
            recip4 = persist.tile([128, S], F32)
            recip4m = persist.tile([128, S], BF16)
            rdram = dramp.tile([4, S], BF16, name="rdram")

            nc.vector.memset(den4[:], 1.0)

            pts = {}

            def emit_x_dma(t):
                xt = xs.tile([128, 8 * 512], BF16, tag="xs", name=f"xs_{t}")
                for d in range(8):
                    nc.gpsimd.dma_start(
                        xt[:, d * 512 : (d + 1) * 512],
                        xT[d * 128 : (d + 1) * 128, t * 512 : (t + 1) * 512],
                    )
                return xt

            def emit_qkv(t, xt):
                for cc in range(6):
                    ps = gp_ps.tile([128, 512], F32, tag="gp", name=f"qkvps_{t}_{cc}")
                    for d in range(8):
                        nc.tensor.matmul(
                            ps[:],
                            wqkv_sb[:, d * LOCAL_COLS + cc * 128 : d * LOCAL_COLS + (cc + 1) * 128],
                            xt[:, d * 512 : (d + 1) * 512],
                            start=(d == 0),
                            stop=(d == 7),
                        )
                    nc.vector.tensor_scalar_add(
                        qkvT[:, cc * S + t * 512 : cc * S + (t + 1) * 512],
                        ps[:],
                        bqkv_sb[:, cc : cc + 1],
                    )

            def emit_vtr(t):
                # V transposes for k-chunks 4t..4t+3, key mask folded in.
                # All 8 transposes of this tile share one psum bank (slots).
                trt = tr_ps.tile([128, 1024], BF16, tag="trps", name=f"trps_{t}")
                for i, sc in enumerate(range(4 * t, 4 * t + 4)):
                    base = sc * VEXT_W
                    for hp in range(2):
                        slot = 2 * i + hp
                        nc.tensor.transpose(
                            trt[:, slot * 128 : (slot + 1) * 128],
                            qkvT[:, (4 + hp) * S + sc * 128 : (4 + hp) * S + (sc + 1) * 128],
                            ident_sb[:],
                        )
                        nc.vector.tensor_scalar_mul(
                            v_ext[:, base + hp * 2 * (HD + 1) : base + (hp * 2 + 2) * (HD + 1)]
                            .rearrange("p (h c) -> p h c", h=2)[:, :, 0:HD],
                            trt[:, slot * 128 : (slot + 1) * 128].rearrange(
                                "p (h c) -> p h c", h=2
                            ),
                            kmask_sb[:, sc : sc + 1],
                        )
                    for h in range(HEADS_PER_CORE):
                        nc.vector.tensor_copy(
                            v_ext[:, base + h * (HD + 1) + HD : base + h * (HD + 1) + HD + 1],
                            kmask_sb[:, sc : sc + 1],
                        )

            def emit_scores_both(t):
                # both head pairs, per 128-wide k-chunk j; heads within a
                # pair run concurrently on disjoint PE row groups
                for j in range(4 * t + 4):
                    db = 128 * (j - 4 * t)  # diag block offset (>=512 => off)
                    for p in range(2):
                        tiles = []
                        for hh in range(2):
                            h = 2 * p + hh
                            sps = sc_ps.tile(
                                [128, 512], F32, tag="scps", name=f"scps_{h}_{t}_{j}"
                            )
                            tiles.append(sps)
                        for hh in range(2):
                            qrow = hh * 64
                            nc.tensor.matmul(
                                tiles[hh][:],
                                qkvT[qrow : qrow + 64, (2 + p) * S + j * 128 : (2 + p) * S + (j + 1) * 128],
                                qkvT[qrow : qrow + 64, p * S + t * 512 : p * S + (t + 1) * 512],
                                start=True,
                                stop=True,
                            )
                        for hh in range(2):
                            h = 2 * p + hh
                            sps = tiles[hh]
                            pt = ptp.tile(
                                [128, 512], BF16, tag="pt", bufs=40, name=f"pt_{h}_{t}_{j}"
                            )
                            if db >= 0:
                                nc.vector.tensor_add(
                                    sps[:, db : db + 128],
                                    sps[:, db : db + 128],
                                    tri_sb[:],
                                )
                                nc.scalar.activation(
                                    pt[:, db:512], sps[:, db:512], AF.Exp, scale=scale
                                )
                                if db > 0:
                                    nc.gpsimd.memset(pt[:, 0:db], 0.0)
                            else:
                                nc.scalar.activation(pt[:], sps[:], AF.Exp, scale=scale)
                            pts[(h, j)] = pt

            def emit_av(p, t):
                jmax = 4 * t + 3
                for hh in range(2):
                    h = 2 * p + hh
                    qrow = hh * 64
                    aps = av_ps.tile(
                        [65, 512], F32, tag="avps", padded_shape=[128, 512],
                        name=f"avps_{h}_{t}",
                    )
                    for j in range(jmax + 1):
                        nc.tensor.matmul(
                            aps[:],
                            v_ext[:, j * VEXT_W + h * (HD + 1) : j * VEXT_W + (h + 1) * (HD + 1)],
                            pts[(h, j)][:],
                            start=(j == 0),
                            stop=(j == jmax),
                        )
                    nc.vector.tensor_scalar_add(
                        den4[h * 32 : h * 32 + 1, t * 512 : (t + 1) * 512],
                        aps[64:65, :],
                        EPS,
                    )
                    nc.vector.tensor_copy(
                        att_u[qrow : qrow + 64, p * S + t * 512 : p * S + (t + 1) * 512],
                        aps[0:64, :],
                    )

            def emit_norm(t):
                cs, ce = t * 512, (t + 1) * 512
                nc.vector.reciprocal_approx_fast(recip4[:, cs:ce], den4[:, cs:ce])
                nc.vector.tensor_mul(
                    recip4m[:, cs:ce], recip4[:, cs:ce], qmask_sb[:, cs:ce]
                )
                nc.sync.dma_start(
                    rdram[:, cs:ce],
                    recip4m[:, cs:ce]
                    .rearrange("(a b) c -> a b c", b=32)[:, 0:1, :]
                    .rearrange("a b c -> (a b) c"),
                )
                for qch in range(2):
                    rr = rrp.tile([128, 512], BF16, tag="rr", name=f"rr_{qch}_{t}")
                    for hh in range(2):
                        h = qch * 2 + hh
                        nc.sync.dma_start(
                            rr[hh * 64 : (hh + 1) * 64, :],
                            rdram[h : h + 1, cs:ce].to_broadcast((64, 512)),
                        )
                    sl = att_u[:, qch * S + cs : qch * S + ce]
                    nc.vector.tensor_mul(sl, sl, rr[:])

            def emit_outproj(t):
                for st in range(4 * t, 4 * t + 4):
                    for n in range(2):
                        ops = gp_ps.tile([128, 512], F32, tag="gp", name=f"outps_{st}_{n}")
                        for ch in range(2):
                            nc.tensor.matmul(
                                ops[:],
                                att_u[:, ch * S + st * 128 : ch * S + (st + 1) * 128],
                                wout_sb[:, ch * D + n * 512 : ch * D + (n + 1) * 512],
                                start=(ch == 0),
                                stop=(ch == 1),
                            )
                        osb = outsb.tile([128, 512], F32, tag="outsb", name=f"outsb_{st}_{n}")
                        nc.scalar.activation(osb[:], ops[:], AF.Identity)
                        nc.sync.dma_start(
                            out[st * 128 : (st + 1) * 128, n * 512 : (n + 1) * 512],
                            osb[:],
                        )

            # ---- software pipeline ----
            xts = {0: emit_x_dma(0)}
            emit_qkv(0, xts[0])
            xts[1] = emit_x_dma(1)
            for t in range(N_TILE):
                emit_vtr(t)
                emit_scores_both(t)
                if t > 0:
                    emit_outproj(t - 1)
                if t < N_TILE - 1:
                    emit_qkv(t + 1, xts[t + 1])
                    if t + 2 < N_TILE:
                        xts[t + 2] = emit_x_dma(t + 2)
                emit_av(0, t)
                emit_av(1, t)
                emit_norm(t)
            emit_outproj(N_TILE - 1)

    from concourse.library_overlay import lower_extended_insts

    lower_extended_insts(nc)
    return _split_waits(nc) if split_waits else nc


def make_in_maps(x, attention_mask, Wqkv, bqkv, Wout):
    """Shard full inputs into the 8 per-core input dicts."""
    import ml_dtypes

    x = np.asarray(x, np.float32)
    attention_mask = np.asarray(attention_mask)
    Wqkv = np.asarray(Wqkv, np.float32)
    bqkv = np.asarray(bqkv, np.float32)
    Wout = np.asarray(Wout, np.float32)

    tri = np.where(
        np.arange(128)[:, None] <= np.arange(128)[None, :], 0.0, NEG
    ).astype(np.float32)
    ident = np.eye(128, dtype=ml_dtypes.bfloat16)

    in_maps = []
    for c in range(CORES):
        b, g = divmod(c, 4)
        cs = 256 * g  # local col start within each of q/k/v blocks
        wq = Wqkv[:, cs : cs + 256]
        wk = Wqkv[:, D + cs : D + cs + 256]
        wv = Wqkv[:, 2 * D + cs : 2 * D + cs + 256]
        w_local = np.ascontiguousarray(
            np.concatenate([wq, wk, wv], axis=1), dtype=ml_dtypes.bfloat16
        )
        b_local = np.concatenate(
            [bqkv[cs : cs + 256], bqkv[D + cs : D + cs + 256], bqkv[2 * D + cs : 2 * D + cs + 256]]
        )
        bqkv_pc = np.ascontiguousarray(b_local.reshape(6, 128).T)
        wout_l = np.ascontiguousarray(Wout[cs : cs + 256, :], dtype=ml_dtypes.bfloat16)
        m = attention_mask[b].astype(np.float32)
        kmask_pc = np.ascontiguousarray(m.reshape(N_KCH, 128).T)
        qmask_rep = np.ascontiguousarray(np.broadcast_to(m[None, :], (128, S)))
        in_maps.append(
            {
                "xT": np.ascontiguousarray(x[b].T, dtype=ml_dtypes.bfloat16),
                "wqkv": w_local,
                "bqkv_pc": bqkv_pc,
                "wout": wout_l,
                "kmask": kmask_pc,
                "qmask_rep": qmask_rep,
                "tri": tri,
                "ident": ident,
            }
        )
    return in_maps


_NC_CACHE = {}


def _get_nc():
    if "nc" not in _NC_CACHE:
        _NC_CACHE["nc"] = build_nc()
    return _NC_CACHE["nc"]


def kernel(x, attention_mask, Wqkv, bqkv, Wout, bout, _trace=False, _trace_kwargs=None):
    bout = np.asarray(bout, np.float32)
    in_maps = make_in_maps(x, attention_mask, Wqkv, bqkv, Wout)
    nc = _get_nc()
    res = run_bass_kernel_spmd(
        nc,
        in_maps,
        list(range(CORES)),
        trace=_trace,
        **(_trace_kwargs or {}),
    )
    outs = [res.results[c]["out"] for c in range(CORES)]
    full = np.empty((B, S, D), np.float32)
    for b in range(B):
        full[b] = outs[4 * b] + outs[4 * b + 1] + outs[4 * b + 2] + outs[4 * b + 3] + bout
    if _trace:
        return full, res
    return full
